# revision 19
# baseline (speedup 1.0000x reference)
"""GAT (2-layer GATConv + BN/ELU + global mean/max pool + 3 FC) on 8 TRN2
NeuronCores via Bass/Tile.

Self-contained: takes FULL inputs (as from setup_inputs()), shards/preps on
host (numpy, index/layout work only), runs one SPMD Bass program on cores
0-7, returns FULL [B, 2] logits.

Strategy (per sharding hint): partition nodes/edges by destination-node
ownership (contiguous 1/8 node ranges), replicate the small weights.  Each
core builds a full per-node feature table in DRAM ([h | a_src | a_dst] rows,
h = x@W.T), then processes its own nodes in degree-sorted tiles of 128:
per-slot indirect-DMA gathers of source rows, softmax over incoming edges
(no max-subtraction -- score ranges are tiny, verified), weighted message
sum, BN+ELU.  Layer-1 outputs are AllGathered (feature-major) so every core
can build the layer-2 table.  Pooling: per-graph sums via one-hot matmul +
AllReduce(add); per-graph max via scatter into a -1e30-padded per-graph-slot
DRAM layout + AllReduce(max).  FC head is computed feature-major on every
core; host takes core 0's output.
"""
import sys

import numpy as np

sys.path.insert(0, "/opt/trn_rl_repo")

import concourse.bass as bass  # noqa: E402
import concourse.tile as tile  # noqa: E402
from concourse import bacc, mybir  # noqa: E402
from concourse.masks import make_identity  # noqa: E402

F32 = mybir.dt.float32
I32 = mybir.dt.int32
AF = mybir.ActivationFunctionType
ALU = mybir.AluOpType

ROW = 80  # table row: h(64) | a_src(8) | a_dst(8)
H, O, HID = 8, 8, 64
NCORES = 8
EPS = 1e-5
NEG = -1e30


# --------------------------------------------------------------------------
# host-side prep: sharding, tiling, index tables, folded weights
# --------------------------------------------------------------------------

def _kron_att(att):
    # A[(h,o), h'] = att[h,o] * delta(h,h')
    A = np.zeros((HID, H), np.float32)
    for h in range(H):
        A[h * O:(h + 1) * O, h] = att[h]
    return A


def host_prep(inp):
    x = np.asarray(inp["x"], np.float32)
    ei = np.asarray(inp["edge_index"], np.int64)
    batch = np.asarray(inp["batch"], np.int64)
    N = x.shape[0]
    NSEG = N // NCORES
    NTIL = (NSEG + 127) // 128
    SEGP = NTIL * 128

    src = np.concatenate([ei[0], np.arange(N, dtype=np.int64)])
    dst = np.concatenate([ei[1], np.arange(N, dtype=np.int64)])
    owner = dst // NSEG

    # global graph counts (for mean)
    cnt = np.bincount(batch, minlength=64).astype(np.float32)
    recip = 1.0 / np.maximum(cnt, 1.0)

    # incoming-edge lists per destination node, self-loop first
    order = np.argsort(dst, kind="stable")
    dsts = dst[order]
    srcs = src[order]
    starts = np.searchsorted(dsts, np.arange(N))
    ends = np.searchsorted(dsts, np.arange(N) + 1)
    deg = (ends - starts).astype(np.int64)  # >= 1 (self loop)

    per_core = []
    for c in range(NCORES):
        lo, hi = c * NSEG, (c + 1) * NSEG
        ldeg = deg[lo:hi]
        perm = np.argsort(-ldeg, kind="stable")  # local ids, degree-desc
        per_core.append({"perm": perm, "deg": ldeg})

    # common tile schedule: K_t = max over cores of max-degree within tile
    Kt = []
    for t in range(NTIL):
        k = 1
        for c in range(NCORES):
            p = per_core[c]["perm"][t * 128:(t + 1) * 128]
            if p.size:
                k = max(k, int(per_core[c]["deg"][p].max()))
        Kt.append(k)
    SK = int(np.sum(Kt))

    # perm-global position of every node: owner seg stride SEGP
    # posg[n] = owner(n)*SEGP + index of n in owner's perm order
    posg = np.empty(N, np.int64)
    for c in range(NCORES):
        perm = per_core[c]["perm"]
        inv = np.empty(NSEG, np.int64)
        inv[perm] = np.arange(NSEG)
        posg[c * NSEG:(c + 1) * NSEG] = c * SEGP + inv
    NT2 = NCORES * SEGP + 1  # + sentinel
    SENT1 = N        # T1 sentinel row
    SENT2 = NT2 - 1  # T2 sentinel row

    # compact active-graph grid for max pooling (per core: GACT slots of
    # SLOT rows; slot j holds the core's j-th nonempty graph)
    percg = np.zeros((NCORES, 64), np.int64)
    for c in range(NCORES):
        bc = batch[c * NSEG:(c + 1) * NSEG]
        percg[c] = np.bincount(bc, minlength=64)[:64]
    gact_per_core = [np.nonzero(percg[c])[0] for c in range(NCORES)]
    GACT = 16
    while max(len(a) for a in gact_per_core) > GACT:
        GACT *= 2
    SLOT = int(percg.max())
    while (GACT * SLOT) % 128:
        SLOT += 1
    GRID = GACT * SLOT
    DUMP = GRID  # row for pad nodes (outside readback range)

    # per-core index tables
    idx1 = np.full((NCORES, 128, SK), SENT1, np.int32)
    idx2 = np.full((NCORES, 128, SK), SENT2, np.int32)
    scat = np.full((NCORES, 128, NTIL), DUMP, np.int32)
    oneh = np.zeros((NCORES, 128, NTIL * 64), np.float32)
    Pmat = np.zeros((NCORES, GACT, 64), np.float32)
    negmask = np.full((NCORES, 64, 64), NEG, np.float32)
    for c in range(NCORES):
        lo = c * NSEG
        perm = per_core[c]["perm"]
        bc = batch[lo:lo + NSEG]
        act = gact_per_core[c]
        slot_of = {int(g): j for j, g in enumerate(act)}
        for j, g in enumerate(act):
            Pmat[c, j, g] = 1.0
            negmask[c, :, g] = 0.0
        # rank within (core, graph) in canonical order
        rank = np.zeros(NSEG, np.int64)
        gc = np.zeros(64, np.int64)
        for i in range(NSEG):
            g = bc[i]
            rank[i] = gc[g]
            gc[g] += 1
        off = 0
        for t in range(NTIL):
            k_t = Kt[t]
            for p in range(128):
                j = t * 128 + p
                if j >= NSEG:
                    break
                n = perm[j]  # local id
                gn = lo + n
                s0, e0 = starts[gn], ends[gn]
                ss = srcs[s0:e0]
                # self-loop first
                selfpos = np.nonzero(ss == gn)[0]
                slots = np.concatenate(
                    [[gn], np.delete(ss, selfpos[0])]) if selfpos.size else ss
                d = slots.shape[0]
                idx1[c, p, off:off + d] = slots
                idx2[c, p, off:off + d] = posg[slots] + 0  # +0: row==pos (sent last)
                g = bc[n]
                scat[c, p, t] = slot_of[int(g)] * SLOT + rank[n]
                oneh[c, p, t * 64 + g] = 1.0
            off += k_t

    # folded weights
    def bnfold(g, b, rm, rv):
        s = np.asarray(g, np.float32) * (1.0 / np.sqrt(np.asarray(rv, np.float32) + EPS))
        t = np.asarray(b, np.float32) - np.asarray(rm, np.float32) * s
        return s, t

    W1 = np.asarray(inp["W1"], np.float32)
    W2 = np.asarray(inp["W2"], np.float32)
    Wcat1 = np.concatenate(
        [W1.T, W1.T @ _kron_att(np.asarray(inp["att_src1"], np.float32)),
         W1.T @ _kron_att(np.asarray(inp["att_dst1"], np.float32))], axis=1)
    Wcat2 = np.concatenate(
        [W2.T, W2.T @ _kron_att(np.asarray(inp["att_src2"], np.float32)),
         W2.T @ _kron_att(np.asarray(inp["att_dst2"], np.float32))], axis=1)

    s1, t1 = bnfold(inp["bn1_g"], inp["bn1_b"], inp["bn1_rm"], inp["bn1_rv"])
    t1f = s1 * np.asarray(inp["b1"], np.float32) + t1
    s2, t2 = bnfold(inp["bn2_g"], inp["bn2_b"], inp["bn2_rm"], inp["bn2_rv"])
    t2f = s2 * np.asarray(inp["b2"], np.float32) + t2

    sf1, tf1 = bnfold(inp["bnf1_g"], inp["bnf1_b"], inp["bnf1_rm"], inp["bnf1_rv"])
    tb1 = sf1 * np.asarray(inp["fc1_b"], np.float32) + tf1
    sf2, tf2 = bnfold(inp["bnf2_g"], inp["bnf2_b"], inp["bnf2_rm"], inp["bnf2_rv"])
    tb2 = sf2 * np.asarray(inp["fc2_b"], np.float32) + tf2

    sentrow = np.concatenate(
        [np.zeros(64, np.float32), np.full(8, NEG, np.float32),
         np.full(8, -NEG, np.float32)])[None, :]

    rep = lambda v: np.tile(np.asarray(v, np.float32)[None, :], (128, 1))
    col = lambda v: np.asarray(v, np.float32)[:, None]

    common = {
        "xT": np.ascontiguousarray(x.T),
        "Wcat1": np.ascontiguousarray(Wcat1),
        "Wcat2": np.ascontiguousarray(Wcat2),
        "sentrow": sentrow,
        "s1rep": rep(s1), "t1rep": rep(t1f),
        "s2rep": rep(s2), "t2rep": rep(t2f),
        "reciprep": np.tile(recip[None, :64], (64, 1)).astype(np.float32),
        "fc1_wT": np.ascontiguousarray(np.asarray(inp["fc1_w"], np.float32).T),
        "fc2_wT": np.ascontiguousarray(np.asarray(inp["fc2_w"], np.float32).T),
        "fc3_wT": np.ascontiguousarray(np.asarray(inp["fc3_w"], np.float32).T),
        "sb1": col(sf1), "tb1": col(tb1),
        "sb2": col(sf2), "tb2": col(tb2),
        "fc3_b": col(inp["fc3_b"]),
    }
    in_maps = []
    for c in range(NCORES):
        m = dict(common)
        m["idx1"] = np.ascontiguousarray(idx1[c])
        m["idx2"] = np.ascontiguousarray(idx2[c])
        m["scat"] = np.ascontiguousarray(scat[c])
        m["oneh"] = np.ascontiguousarray(oneh[c])
        m["Pmat"] = np.ascontiguousarray(Pmat[c])
        m["negmask"] = np.ascontiguousarray(negmask[c])
        in_maps.append(m)

    cfg = dict(N=N, NSEG=NSEG, NTIL=NTIL, SEGP=SEGP, Kt=Kt, SK=SK,
               NT2=NT2, SENT1=SENT1, SENT2=SENT2, SLOT=SLOT,
               GACT=GACT, GRID=GRID)
    return cfg, in_maps


# --------------------------------------------------------------------------
# device program
# --------------------------------------------------------------------------

def build_program(cfg):
    N, NSEG, NTIL, SEGP = cfg["N"], cfg["NSEG"], cfg["NTIL"], cfg["SEGP"]
    Kt, SK, NT2 = cfg["Kt"], cfg["SK"], cfg["NT2"]
    SLOT, GACT, GRID = cfg["SLOT"], cfg["GACT"], cfg["GRID"]
    Kmax = max(Kt)

    nc = bacc.Bacc(None, target_bir_lowering=False)
    nc.num_devices = NCORES

    # I/O
    xT = nc.dram_tensor("xT", [128, N], F32, kind="ExternalInput")
    Wc1 = nc.dram_tensor("Wcat1", [128, ROW], F32, kind="ExternalInput")
    Wc2 = nc.dram_tensor("Wcat2", [HID, ROW], F32, kind="ExternalInput")
    sentrow = nc.dram_tensor("sentrow", [1, ROW], F32, kind="ExternalInput")
    idx1 = nc.dram_tensor("idx1", [128, SK], I32, kind="ExternalInput")
    idx2 = nc.dram_tensor("idx2", [128, SK], I32, kind="ExternalInput")
    scat = nc.dram_tensor("scat", [128, NTIL], I32, kind="ExternalInput")
    oneh = nc.dram_tensor("oneh", [128, NTIL * 64], F32, kind="ExternalInput")
    Pmat = nc.dram_tensor("Pmat", [GACT, 64], F32, kind="ExternalInput")
    negmask = nc.dram_tensor("negmask", [64, 64], F32, kind="ExternalInput")
    s1rep = nc.dram_tensor("s1rep", [128, HID], F32, kind="ExternalInput")
    t1rep = nc.dram_tensor("t1rep", [128, HID], F32, kind="ExternalInput")
    s2rep = nc.dram_tensor("s2rep", [128, HID], F32, kind="ExternalInput")
    t2rep = nc.dram_tensor("t2rep", [128, HID], F32, kind="ExternalInput")
    reciprep = nc.dram_tensor("reciprep", [64, 64], F32, kind="ExternalInput")
    fc1_wT = nc.dram_tensor("fc1_wT", [128, 64], F32, kind="ExternalInput")
    fc2_wT = nc.dram_tensor("fc2_wT", [64, 32], F32, kind="ExternalInput")
    fc3_wT = nc.dram_tensor("fc3_wT", [32, 2], F32, kind="ExternalInput")
    sb1 = nc.dram_tensor("sb1", [64, 1], F32, kind="ExternalInput")
    tb1 = nc.dram_tensor("tb1", [64, 1], F32, kind="ExternalInput")
    sb2 = nc.dram_tensor("sb2", [32, 1], F32, kind="ExternalInput")
    tb2 = nc.dram_tensor("tb2", [32, 1], F32, kind="ExternalInput")
    fc3_b = nc.dram_tensor("fc3_b", [2, 1], F32, kind="ExternalInput")
    out = nc.dram_tensor("logitsT", [2, 64], F32, kind="ExternalOutput")

    # internal DRAM
    T1 = nc.dram_tensor("T1", [N + 1, ROW], F32)
    T2 = nc.dram_tensor("T2", [NT2, ROW], F32)
    H2pad = nc.dram_tensor("H2pad", [GRID + 128, HID], F32)
    cc_h1_in = nc.dram_tensor("cc_h1_in", [HID, SEGP], F32)
    cc_h1_out = nc.dram_tensor("cc_h1_out", [NCORES, HID, SEGP], F32,
                               addr_space="Shared")
    cc_sum_in = nc.dram_tensor("cc_sum_in", [64, 64], F32)
    cc_sum_out = nc.dram_tensor("cc_sum_out", [64, 64], F32, addr_space="Shared")
    cc_max_in = nc.dram_tensor("cc_max_in", [64, 64], F32)
    cc_max_out = nc.dram_tensor("cc_max_out", [64, 64], F32, addr_space="Shared")

    RG = [list(range(NCORES))]

    with tile.TileContext(nc) as tc:
        import contextlib
        ctx = contextlib.ExitStack()
        with ctx:
            cons = ctx.enter_context(tc.tile_pool(name="cons", bufs=1))
            xin = ctx.enter_context(tc.tile_pool(name="xin", bufs=2))
            stag = ctx.enter_context(tc.tile_pool(name="stag", bufs=2))
            psb = ctx.enter_context(tc.tile_pool(name="psb", bufs=2, space="PSUM"))
            pool_ps = ctx.enter_context(
                tc.tile_pool(name="pool_ps", bufs=1, space="PSUM"))
            idxp = ctx.enter_context(tc.tile_pool(name="idxp", bufs=2))
            gat = ctx.enter_context(tc.tile_pool(name="gat", bufs=2))
            work = ctx.enter_context(tc.tile_pool(name="work", bufs=2))
            outp = ctx.enter_context(tc.tile_pool(name="outp", bufs=2))
            big = ctx.enter_context(tc.tile_pool(name="big", bufs=1))

            # ---- constants in SBUF
            wc1 = cons.tile([128, ROW], F32)
            nc.sync.dma_start(wc1[:], Wc1[:])
            wc2 = cons.tile([HID, ROW], F32)
            nc.sync.dma_start(wc2[:], Wc2[:])
            ident = cons.tile([128, 128], F32)
            make_identity(nc, ident[:])
            s1t = cons.tile([128, HID], F32)
            nc.sync.dma_start(s1t[:], s1rep[:])
            t1t = cons.tile([128, HID], F32)
            nc.sync.dma_start(t1t[:], t1rep[:])
            s2t = cons.tile([128, HID], F32)
            nc.sync.dma_start(s2t[:], s2rep[:])
            t2t = cons.tile([128, HID], F32)
            nc.sync.dma_start(t2t[:], t2rep[:])
            onehot_sb = cons.tile([128, NTIL * 64], F32)
            nc.sync.dma_start(onehot_sb[:], oneh[:])
            scat_sb = cons.tile([128, NTIL], I32)
            nc.sync.dma_start(scat_sb[:], scat[:])

            # sentinel rows
            sent_sb = cons.tile([1, ROW], F32)
            nc.sync.dma_start(sent_sb[:], sentrow[:])
            nc.sync.dma_start(T1[N:N + 1, :], sent_sb[:])
            nc.sync.dma_start(T2[NT2 - 1:NT2, :], sent_sb[:])

            # ---- phase A: build T1 (canonical rows) ----
            BLK = 8  # chunks per staging flush
            def build_table(tbl, n_rows, lhs_src, wtile):
                # lhs_src(c, cw) -> AP [kdim, cw] for chunk c
                nfull = n_rows // 128
                rem = n_rows - nfull * 128
                c = 0
                while c < nfull:
                    grp = min(BLK, nfull - c)
                    st = stag.tile([128, BLK * ROW], F32, tag="tstag")
                    for j in range(grp):
                        ps = psb.tile([128, ROW], F32, tag="ps")
                        nc.tensor.matmul(ps[:], lhsT=lhs_src(c + j, 128),
                                         rhs=wtile[:], start=True, stop=True)
                        nc.vector.tensor_copy(
                            st[:, j * ROW:(j + 1) * ROW], ps[:])
                    dst = tbl[c * 128:(c + grp) * 128, :].rearrange(
                        "(j p) r -> p j r", p=128)
                    nc.sync.dma_start(
                        dst, st[:, :grp * ROW].rearrange(
                            "p (j r) -> p j r", r=ROW))
                    c += grp
                if rem:
                    ps = psb.tile([128, ROW], F32, tag="ps")
                    nc.tensor.matmul(ps[:rem, :], lhsT=lhs_src(nfull, rem),
                                     rhs=wtile[:], start=True, stop=True)
                    st = stag.tile([128, BLK * ROW], F32, tag="tstag")
                    nc.vector.tensor_copy(st[:rem, :ROW], ps[:rem, :])
                    nc.sync.dma_start(tbl[nfull * 128:n_rows, :],
                                      st[:rem, :ROW])

            XBLK = 1024
            xbufs = {}
            def lhs1(c, cw):
                blk = (c * 128) // XBLK
                if blk not in xbufs:
                    xb = xin.tile([128, max(XBLK, SEGP)], F32, tag="stream")
                    w = min(XBLK, N - blk * XBLK)
                    nc.sync.dma_start(xb[:, :w], xT[:, blk * XBLK:blk * XBLK + w])
                    xbufs.clear()
                    xbufs[blk] = xb
                off = c * 128 - blk * XBLK
                return xbufs[blk][:, off:off + cw]

            build_table(T1, N, lhs1, wc1)

            # ---- phase B/E shared edge layer ----
            def edge_layer(tbl, idx_in, s_t, t_t, sink):
                off = 0
                for t in range(NTIL):
                    K = Kt[t]
                    it = idxp.tile([128, Kmax], I32, tag="it")
                    nc.sync.dma_start(it[:, :K], idx_in[:, off:off + K])
                    G = gat.tile([128, Kmax * ROW], F32, tag="G")
                    for k in range(K):
                        nc.gpsimd.indirect_dma_start(
                            out=G[:, k * ROW:(k + 1) * ROW],
                            out_offset=None,
                            in_=tbl[:],
                            in_offset=bass.IndirectOffsetOnAxis(
                                ap=it[:, k:k + 1], axis=0))
                    Gv = G[:, :K * ROW].rearrange("p (k r) -> p k r", r=ROW)
                    epre = work.tile([128, Kmax * 8], F32, tag="epre")
                    epv = epre[:, :K * 8].rearrange("p (k h) -> p k h", h=8)
                    adst = Gv[:, 0:1, 72:80].broadcast_to([128, K, 8])
                    nc.vector.tensor_tensor(out=epv, in0=Gv[:, :, 64:72],
                                            in1=adst, op=ALU.add)
                    nc.vector.scalar_tensor_tensor(
                        out=epre[:, :K * 8], in0=epre[:, :K * 8], scalar=0.2,
                        in1=epre[:, :K * 8], op0=ALU.mult, op1=ALU.max)
                    nc.scalar.activation(epre[:, :K * 8], epre[:, :K * 8], AF.Exp)
                    den = work.tile([128, 8], F32, tag="den")
                    nc.vector.tensor_reduce(out=den[:], in_=epv.transpose([0, 2, 1]),
                                            axis=mybir.AxisListType.X, op=ALU.add)
                    rden = work.tile([128, 8], F32, tag="rden")
                    nc.vector.reciprocal(rden[:], den[:])
                    exb = epv.unsqueeze(3).broadcast_to([128, K, 8, 8])
                    hv = Gv[:, :, 0:64].rearrange("p k (h o) -> p k h o", h=8, o=8)
                    nc.vector.tensor_tensor(out=hv, in0=hv, in1=exb, op=ALU.mult)
                    y = outp.tile([128, 64], F32, tag="y")
                    yv = y[:].rearrange("p (h o) -> p h o", h=8, o=8)
                    nc.vector.tensor_reduce(out=yv, in_=hv.transpose([0, 2, 3, 1]),
                                            axis=mybir.AxisListType.X, op=ALU.add)
                    rdb = rden[:].unsqueeze(2).broadcast_to([128, 8, 8])
                    nc.vector.tensor_tensor(out=yv, in0=yv, in1=rdb, op=ALU.mult)
                    # y = elu(s*y + t):  m=min(z,0); elu = relu(z) + exp(m) - 1
                    nc.vector.tensor_tensor(out=y[:], in0=y[:], in1=s_t[:, :64],
                                            op=ALU.mult)
                    nc.vector.tensor_tensor(out=y[:], in0=y[:], in1=t_t[:, :64],
                                            op=ALU.add)
                    m = work.tile([128, 64], F32, tag="m")
                    nc.vector.tensor_scalar_min(m[:], y[:], 0.0)
                    nc.scalar.activation(m[:], m[:], AF.Exp)
                    nc.scalar.activation(y[:], y[:], AF.Relu)
                    nc.vector.scalar_tensor_tensor(
                        out=y[:], in0=m[:], scalar=-1.0, in1=y[:],
                        op0=ALU.add, op1=ALU.add)
                    sink(t, y)
                    off += K

            # L1 sink: transpose into h1T
            h1T = big.tile([HID, SEGP], F32, tag="bigT")
            nc.vector.memset(h1T[:], 0.0)

            def sink1(t, y):
                psT = psb.tile([64, 128], F32, tag="ps")
                nc.tensor.transpose(psT[:], y[:], ident[:])
                nc.vector.tensor_copy(h1T[:, t * 128:(t + 1) * 128], psT[:])

            edge_layer(T1, idx1, s1t, t1t, sink1)

            # ---- phase C: allgather h1T ----
            nc.sync.dma_start(cc_h1_in[:], h1T[:])
            nc.gpsimd.collective_compute(
                "AllGather", ALU.bypass, replica_groups=RG,
                ins=[cc_h1_in[:]], outs=[cc_h1_out[:]])

            # ---- phase D: build T2 from gathered h1T ----
            segbufs = {}
            def lhs2(c, cw):
                s = (c * 128) // SEGP
                if s not in segbufs:
                    sb = xin.tile([HID, max(XBLK, SEGP)], F32, tag="stream")
                    nc.sync.dma_start(sb[:, :SEGP], cc_h1_out[s])
                    segbufs.clear()
                    segbufs[s] = sb
                off = c * 128 - s * SEGP
                return segbufs[s][:, off:off + cw]

            build_table(T2, NCORES * SEGP, lhs2, wc2)

            # ---- phase E0: init H2pad to NEG ----
            neg = stag.tile([128, 2048], F32, tag="negf")
            nc.vector.memset(neg[:], NEG)
            total = (GRID + 128) * HID
            CH = 128 * 2048
            nflush = (total + CH - 1) // CH
            flat = H2pad[:].rearrange("n d -> (n d)")
            for i in range(nflush):
                w = min(CH, total - i * CH)
                rows = w // 2048
                nc.sync.dma_start(
                    flat[i * CH:i * CH + w].rearrange("(p m) -> p m", p=rows),
                    neg[:rows, :])

            # ---- phase E: layer 2 + scatter + pooled sums ----
            pool_acc = pool_ps.tile([64, 64], F32)

            def sink2(t, y):
                nc.gpsimd.indirect_dma_start(
                    out=H2pad[:], out_offset=bass.IndirectOffsetOnAxis(
                        ap=scat_sb[:, t:t + 1], axis=0),
                    in_=y[:], in_offset=None)
                nc.tensor.matmul(pool_acc[:], lhsT=onehot_sb[:, t * 64:(t + 1) * 64],
                                 rhs=y[:], start=(t == 0), stop=(t == NTIL - 1))

            edge_layer(T2, idx2, s2t, t2t, sink2)

            # ---- phase F: pooling ----
            sums_sb = stag.tile([64, 64], F32, tag="sums")
            nc.vector.tensor_copy(sums_sb[:], pool_acc[:])
            nc.sync.dma_start(cc_sum_in[:], sums_sb[:])
            nc.gpsimd.collective_compute(
                "AllReduce", ALU.add, replica_groups=RG,
                ins=[cc_sum_in[:]], outs=[cc_sum_out[:]])

            # readback compact grid, transpose, per-active-slot max,
            # then map active slots -> global graphs via Pmat + negmask
            h2cmpT = big.tile([HID, GRID], F32, tag="bigT")
            NCH = GRID // 128
            for cchunk in range(NCH):
                hb = stag.tile([128, HID], F32, tag="hb")
                nc.sync.dma_start(hb[:], H2pad[cchunk * 128:(cchunk + 1) * 128, :])
                psT = psb.tile([64, 128], F32, tag="ps")
                nc.tensor.transpose(psT[:], hb[:], ident[:])
                nc.vector.tensor_copy(h2cmpT[:, cchunk * 128:(cchunk + 1) * 128],
                                      psT[:])
            Lmax = stag.tile([64, GACT], F32, tag="Lmax")
            nc.vector.tensor_reduce(
                out=Lmax[:], in_=h2cmpT[:].rearrange("f (j s) -> f j s", s=SLOT),
                axis=mybir.AxisListType.X, op=ALU.max)
            LT_ps = psb.tile([GACT, 64], F32, tag="ps")
            nc.tensor.transpose(LT_ps[:], Lmax[:], ident[0:64, 0:64])
            LT = stag.tile([GACT, 64], F32, tag="LT")
            nc.vector.tensor_copy(LT[:], LT_ps[:])
            pm = cons.tile([GACT, 64], F32)
            nc.sync.dma_start(pm[:], Pmat[:])
            nm = cons.tile([64, 64], F32)
            nc.sync.dma_start(nm[:], negmask[:])
            mx_ps = psb.tile([64, 64], F32, tag="ps")
            nc.tensor.matmul(mx_ps[:], lhsT=LT[:], rhs=pm[:], start=True,
                             stop=True)
            maxsT = stag.tile([64, 64], F32, tag="maxs")
            nc.vector.tensor_tensor(out=maxsT[:], in0=mx_ps[:], in1=nm[:],
                                    op=ALU.add)
            nc.sync.dma_start(cc_max_in[:], maxsT[:])
            nc.gpsimd.collective_compute(
                "AllReduce", ALU.max, replica_groups=RG,
                ins=[cc_max_in[:]], outs=[cc_max_out[:]])

            # assemble gT [128 feat, 64 graphs]
            sumsG = stag.tile([64, 64], F32, tag="sumsG")
            nc.sync.dma_start(sumsG[:], cc_sum_out[:])
            psT2 = psb.tile([64, 64], F32, tag="ps")
            nc.tensor.transpose(psT2[:], sumsG[:], ident[0:64, 0:64])  # -> [64f,64g]
            rc = cons.tile([64, 64], F32)
            nc.sync.dma_start(rc[:], reciprep[:])
            gT = big.tile([128, 64], F32)
            nc.vector.tensor_tensor(out=gT[0:64, :], in0=psT2[:], in1=rc[:],
                                    op=ALU.mult)
            maxr = stag.tile([64, 64], F32, tag="maxr")
            nc.sync.dma_start(maxr[:], cc_max_out[:])
            nc.vector.tensor_copy(gT[64:128, :], maxr[:])

            # ---- phase G: FC head (feature-major) ----
            w1 = cons.tile([128, 64], F32)
            nc.sync.dma_start(w1[:], fc1_wT[:])
            w2 = cons.tile([64, 32], F32)
            nc.sync.dma_start(w2[:], fc2_wT[:])
            w3 = cons.tile([32, 2], F32)
            nc.sync.dma_start(w3[:], fc3_wT[:])
            a1 = cons.tile([64, 1], F32)
            nc.sync.dma_start(a1[:], sb1[:])
            b1t = cons.tile([64, 1], F32)
            nc.sync.dma_start(b1t[:], tb1[:])
            a2 = cons.tile([32, 1], F32)
            nc.sync.dma_start(a2[:], sb2[:])
            b2t = cons.tile([32, 1], F32)
            nc.sync.dma_start(b2t[:], tb2[:])
            b3 = cons.tile([2, 1], F32)
            nc.sync.dma_start(b3[:], fc3_b[:])

            z1 = psb.tile([64, 64], F32, tag="ps")
            nc.tensor.matmul(z1[:], lhsT=w1[:], rhs=gT[:], start=True, stop=True)
            y1 = stag.tile([64, 64], F32, tag="y1")
            nc.scalar.activation(y1[:], z1[:], AF.Relu, bias=b1t[:], scale=a1[:])
            z2 = psb.tile([32, 64], F32, tag="ps")
            nc.tensor.matmul(z2[:], lhsT=w2[:], rhs=y1[:], start=True, stop=True)
            y2f = stag.tile([32, 64], F32, tag="y2f")
            nc.scalar.activation(y2f[:], z2[:], AF.Relu, bias=b2t[:], scale=a2[:])
            z3 = psb.tile([2, 64], F32, tag="ps")
            nc.tensor.matmul(z3[:], lhsT=w3[:], rhs=y2f[:], start=True, stop=True)
            lg = stag.tile([2, 64], F32, tag="lg")
            nc.scalar.activation(lg[:], z3[:], AF.Identity, bias=b3[:])
            nc.sync.dma_start(out[:], lg[:])

    nc.compile()
    return nc


# --------------------------------------------------------------------------
# entry point
# --------------------------------------------------------------------------

def kernel(**inputs):
    cfg, in_maps = host_prep(inputs)
    nc = build_program(cfg)
    from concourse.bass_utils import run_bass_kernel_spmd
    r = run_bass_kernel_spmd(nc, in_maps, list(range(NCORES)))
    logitsT = r.results[0]["logitsT"]
    return np.ascontiguousarray(logitsT.T.astype(np.float32))
